# revision 5
# baseline (speedup 1.0000x reference)
"""Trainium2 Bass kernel for the AttnBlock-style attention module.

Reference computation (note softmax over axis=1, the *i* axis):
    q = wq @ x + bq ; k = wk @ x + bk ; v = wv @ x + bv      (per-pixel 1x1 conv)
    s[b,i,j] = (q[b,:,i] . k[b,:,j]) * C**-0.5
    attn = softmax_i(s)                                      (normalize over i!)
    out[b,c,i] = sum_j attn[b,i,j] v[b,c,j]
    y = wp @ out + bp

Sharding: 8 cores = 4 batches x 2 j-halves. The softmax over i is local to a
j-split (it normalizes each attention *column* j over all i). Each core gets x
with its j-half rotated to columns 0..2047 (a pure permutation of the pixel
axis, which passes through every per-pixel op and the i-softmax unchanged; the
host un-rotates the partial output). Each core:
  - computes q for all N=4096 pixels, k/v for columns 0..2047,
  - s_T[j, i] = k^T q   (j on partitions -> softmax reduction is free-axis),
  - attn = exp(s/16) stored unnormalized in bf16; per-j denominators D[j]
    from the fused activation accum_out; 1/D folded into v rows,
  - out_partial[c, i] = sum_{j in half} v_scaled[c,j] attn_T[j,i],
  - y_partial = wp @ out_partial   (bias bp added on host).
Host un-rotates and sums the two j-half partials per batch and adds bp.
"""

import numpy as np

import concourse.bass as bass
import concourse.mybir as mybir
import concourse.tile as tile
from concourse import bacc
from concourse import bass_utils

P = 128
B = 4
C = 256
N = 4096          # 64*64 pixels
NJ = 2048         # j columns per core
NJT = NJ // P     # 16 j tiles
SCALE = 1.0 / np.sqrt(C).item()   # 1/16

F32 = mybir.dt.float32
BF16 = mybir.dt.bfloat16
F32R = mybir.dt.float32r
AF = mybir.ActivationFunctionType


def _fr(ap):
    # fp32 data, float32r matmul mode: full PE rate when free dim >= 256.
    return ap.bitcast(F32R)


def _build_module():
    nc = bacc.Bacc("TRN2", target_bir_lowering=False, debug=False, num_devices=8)

    x_t = nc.dram_tensor("x", [C, N], F32R, kind="ExternalInput")
    w_t = nc.dram_tensor("wT", [4, C, C], F32R, kind="ExternalInput")  # q,k,v,p (transposed)
    b_t = nc.dram_tensor("b", [2, C], F32, kind="ExternalInput")      # bq, bk
    bv_t = nc.dram_tensor("bv", [1, C], F32, kind="ExternalInput")
    y_t = nc.dram_tensor("y", [C, N], F32, kind="ExternalOutput")

    with tile.TileContext(nc) as tc:
        _emit(nc, tc, x_t, w_t, b_t, bv_t, y_t)
    nc.compile()
    return nc


def _emit(nc, tc, x_t, w_t, b_t, bv_t, y_t):
    from contextlib import ExitStack

    with ExitStack() as top:
        const = top.enter_context(tc.tile_pool(name="const", bufs=1))
        big = top.enter_context(tc.tile_pool(name="big", bufs=1))

        # ---- constants (packed to dodge the 4KB alloc granularity) -----
        # w_all[:, 2*w + ci, :] = rows ci*128.. of weight w's transpose [ci, co]
        w_all = const.tile([P, 8, C], F32R, tag="w_all", name="w_all")
        for w in range(4):
            for ci in range(2):
                nc.sync.dma_start(
                    w_all[:, 2 * w + ci, :],
                    w_t.ap()[w, ci * P:(ci + 1) * P, :],
                )

        def wslice(w, ci, ch):   # lhsT [128 ci, 128 co] for co half ch
            return w_all[:, 2 * w + ci, ch * P:(ch + 1) * P]

        # b_all columns: 0,1 = bq halves; 2,3 = bk halves
        b_all = const.tile([P, 4], F32, tag="b_all", name="b_all")
        for w in range(2):
            for ch in range(2):
                nc.sync.dma_start(
                    b_all[:, 2 * w + ch: 2 * w + ch + 1],
                    b_t.ap()[w, ch * P:(ch + 1) * P].rearrange("(p o) -> p o", o=1),
                )
        bv_sb = const.tile([P, C], F32, tag="bv", name="bv_sb")
        nc.sync.dma_start(
            bv_sb[:], bass.AP(tensor=bv_t, offset=0, ap=[[0, P], [1, C]])
        )

        # ---- persistent activations -----------------------------------
        q_bf = [big.tile([P, N], BF16, tag=f"q{ch}", name=f"q{ch}") for ch in range(2)]
        k_bf = [big.tile([P, NJ], BF16, tag=f"k{ch}", name=f"k{ch}") for ch in range(2)]
        v_all = big.tile([P, NJT, C], BF16, tag="v_all", name="v_all")
        attn = [big.tile([P, N], BF16, tag=f"a{jt}", name=f"a{jt}") for jt in range(NJT)]
        # d_all columns: 0:32 = per-(jt,ih) partial sums, 32:48 = D, 48:64 = 1/D
        d_all = big.tile([P, 64], F32, tag="d_all", name="d_all")

        # ---- phase 1: QKV projections ---------------------------------
        with tc.tile_pool(name="xload", bufs=1) as xp, \
             tc.tile_pool(name="ps_qkv", bufs=8, space="PSUM") as pq:
            x_sb = []
            for ci in range(2):
                xt = xp.tile([P, N], F32R, tag=f"x{ci}", name=f"x{ci}")
                nc.sync.dma_start(xt[:], x_t.ap()[ci * P:(ci + 1) * P, :])
                x_sb.append(xt)

            # q[co, i] = wqT.T @ x  (+bq) -> bf16
            for ch in range(2):
                for icg in range(2):
                    pss = [pq.tile([P, 512], F32, tag="ps", name="ps") for _ in range(4)]
                    for ci in range(2):
                        lhs = wslice(0, ci, ch)
                        for t in range(4):
                            ic = icg * 4 + t
                            nc.tensor.matmul(
                                pss[t][:], lhs,
                                x_sb[ci][:, ic * 512:(ic + 1) * 512],
                                start=(ci == 0), stop=(ci == 1),
                            )
                    for t in range(4):
                        ic = icg * 4 + t
                        nc.scalar.activation(
                            q_bf[ch][:, ic * 512:(ic + 1) * 512], pss[t][:],
                            AF.Identity, bias=b_all[:, ch:ch + 1],
                        )

            # k[co, j] over columns 0..NJ (+bk) -> bf16
            for ch in range(2):
                pss = [pq.tile([P, 512], F32, tag="ps", name="ps") for _ in range(4)]
                for ci in range(2):
                    lhs = wslice(1, ci, ch)
                    for t in range(4):
                        nc.tensor.matmul(
                            pss[t][:], lhs,
                            x_sb[ci][:, t * 512:(t + 1) * 512],
                            start=(ci == 0), stop=(ci == 1),
                        )
                for t in range(4):
                    nc.scalar.activation(
                        k_bf[ch][:, t * 512:(t + 1) * 512], pss[t][:],
                        AF.Identity, bias=b_all[:, 2 + ch:3 + ch],
                    )

            # v_T[j, co] = x[:, :NJ]^T @ wvT (+bv broadcast) -> bf16
            for jtg in range(4):
                pss = [pq.tile([P, C], F32, tag="ps", name="ps") for _ in range(4)]
                for ci in range(2):
                    for t in range(4):
                        jt = jtg * 4 + t
                        nc.tensor.matmul(
                            pss[t][:],
                            x_sb[ci][:, jt * P:(jt + 1) * P],
                            w_all[:, 2 * 2 + ci, :],
                            start=(ci == 0), stop=(ci == 1),
                        )
                for t in range(4):
                    nc.vector.tensor_add(
                        v_all[:, jtg * 4 + t, :], pss[t][:], bv_sb[:]
                    )

        # ---- phase 2: scores + exp + denominators ---------------------
        with tc.tile_pool(name="ps_s", bufs=2, space="PSUM") as psp:
            for jt in range(NJT):
                for ih in range(2):
                    ps = psp.tile([P, 2048], F32, tag="s", name="s_ps")
                    for ch in range(2):
                        lhs = k_bf[ch][:, jt * P:(jt + 1) * P]
                        for t in range(4):
                            nc.tensor.matmul(
                                ps[:, t * 512:(t + 1) * 512], lhs,
                                q_bf[ch][:, ih * 2048 + t * 512: ih * 2048 + (t + 1) * 512],
                                start=(ch == 0), stop=(ch == 1),
                            )
                    col = jt * 2 + ih
                    nc.scalar.activation(
                        attn[jt][:, ih * 2048:(ih + 1) * 2048], ps[:],
                        AF.Exp, scale=float(SCALE),
                        accum_out=d_all[:, col:col + 1],
                    )

        # per-j denominator: D = sum_i exp(s); fold 1/D into v rows
        nc.vector.reduce_sum(
            d_all[:, 32:48], d_all[:, 0:32].rearrange("p (a b) -> p a b", b=2),
            axis=mybir.AxisListType.X,
        )
        nc.vector.reciprocal(d_all[:, 48:64], d_all[:, 32:48])
        for jt in range(NJT):
            nc.vector.tensor_scalar_mul(
                v_all[:, jt, :], v_all[:, jt, :], d_all[:, 48 + jt:49 + jt]
            )

        # ---- phase 3: out = v_scaled @ attn ; y = wp @ out -------------
        with tc.tile_pool(name="ps_o", bufs=1, space="PSUM") as po, \
             tc.tile_pool(name="ps_y", bufs=1, space="PSUM") as py, \
             tc.tile_pool(name="osb", bufs=2) as osb_pool, \
             tc.tile_pool(name="ysb", bufs=2) as ysb_pool:
            for iq in range(4):
                o_ps = [po.tile([P, 1024], F32, tag=f"o{ch}", name=f"o{ch}") for ch in range(2)]
                for jt in range(NJT):
                    for ch in range(2):
                        lhs = v_all[:, jt, ch * P:(ch + 1) * P]
                        for t in range(2):
                            nc.tensor.matmul(
                                o_ps[ch][:, t * 512:(t + 1) * 512], lhs,
                                attn[jt][:, iq * 1024 + t * 512: iq * 1024 + (t + 1) * 512],
                                start=(jt == 0), stop=(jt == NJT - 1),
                            )
                o_sb = [osb_pool.tile([P, 1024], F32R, tag=f"osb{ch}", name=f"osb{ch}") for ch in range(2)]
                for ch in range(2):
                    nc.scalar.copy(o_sb[ch][:], o_ps[ch][:])
                for cho in range(2):
                    y_ps = py.tile([P, 1024], F32, tag=f"y{cho}", name=f"y{cho}")
                    for ci in range(2):
                        lhs = wslice(3, ci, cho)
                        for t in range(2):
                            nc.tensor.matmul(
                                y_ps[:, t * 512:(t + 1) * 512], lhs,
                                o_sb[ci][:, t * 512:(t + 1) * 512],
                                start=(ci == 0), stop=(ci == 1),
                            )
                    y_sb = ysb_pool.tile([P, 1024], F32, tag=f"ysb{cho}", name=f"ysb{cho}")
                    nc.vector.tensor_copy(y_sb[:], y_ps[:])
                    nc.sync.dma_start(
                        y_t.ap()[cho * P:(cho + 1) * P, iq * 1024:(iq + 1) * 1024],
                        y_sb[:],
                    )


_nc_cache = None
LAST_EXEC_TIME_NS = None


def _get_nc():
    global _nc_cache
    if _nc_cache is None:
        _nc_cache = _build_module()
    return _nc_cache


def kernel(x, wq, bq, wk, bk, wv, bv, wp, bp):
    global LAST_EXEC_TIME_NS
    nc = _get_nc()

    x = np.asarray(x, dtype=np.float32).reshape(B, C, N)
    wT = np.ascontiguousarray(np.stack([
        np.asarray(w, dtype=np.float32).T for w in (wq, wk, wv, wp)
    ]))
    b2 = np.ascontiguousarray(np.stack([
        np.asarray(bq, dtype=np.float32), np.asarray(bk, dtype=np.float32)
    ]))
    bv2 = np.ascontiguousarray(np.asarray(bv, dtype=np.float32).reshape(1, C))
    bp1 = np.asarray(bp, dtype=np.float32).reshape(C)

    in_maps = []
    for core in range(8):
        b, h = divmod(core, 2)
        xb = x[b] if h == 0 else np.ascontiguousarray(np.roll(x[b], -NJ, axis=1))
        in_maps.append({"x": xb, "wT": wT, "b": b2, "bv": bv2})

    res = bass_utils.run_bass_kernel_spmd(nc, in_maps, core_ids=list(range(8)))
    if res.exec_time_ns is not None:
        LAST_EXEC_TIME_NS = res.exec_time_ns

    y = np.zeros((B, C, N), np.float32)
    for b in range(B):
        y[b] = res.results[2 * b]["y"] + np.roll(res.results[2 * b + 1]["y"], NJ, axis=1)
    y += bp1.reshape(1, C, 1)
    return y.reshape(B, C, 64, 64)


# revision 6
# speedup vs baseline: 1.0284x; 1.0284x over previous
"""Trainium2 Bass kernel for the AttnBlock-style attention module.

Reference computation (note softmax over axis=1, the *i* axis):
    q = wq @ x + bq ; k = wk @ x + bk ; v = wv @ x + bv      (per-pixel 1x1 conv)
    s[b,i,j] = (q[b,:,i] . k[b,:,j]) * C**-0.5
    attn = softmax_i(s)                                      (normalize over i!)
    out[b,c,i] = sum_j attn[b,i,j] v[b,c,j]
    y = wp @ out + bp

Sharding: 8 cores = 4 batches x 2 j-halves. The softmax over i is local to a
j-split (it normalizes each attention *column* j over all i). Each core gets x
with its j-half rotated to columns 0..2047 (a pure permutation of the pixel
axis, which passes through every per-pixel op and the i-softmax unchanged; the
host un-rotates the partial output). Each core:
  - computes q for all N=4096 pixels, k/v for columns 0..2047,
  - s_T[j, i] = k^T q   (j on partitions -> softmax reduction is free-axis),
  - attn = exp(s/16) stored unnormalized in bf16; per-j denominators D[j]
    from the fused activation accum_out; 1/D folded into v rows,
  - out_partial[c, i] = sum_{j in half} v_scaled[c,j] attn_T[j,i],
  - y_partial = wp @ out_partial   (bias bp added on host).
Host un-rotates and sums the two j-half partials per batch and adds bp.
"""

import numpy as np

import concourse.bass as bass
import concourse.mybir as mybir
import concourse.tile as tile
from concourse import bacc
from concourse import bass_utils

P = 128
B = 4
C = 256
N = 4096          # 64*64 pixels
NJ = 2048         # j columns per core
NJT = NJ // P     # 16 j tiles
SCALE = 1.0 / np.sqrt(C).item()   # 1/16

F32 = mybir.dt.float32
BF16 = mybir.dt.bfloat16
F32R = mybir.dt.float32r
AF = mybir.ActivationFunctionType


def _fr(ap):
    # fp32 data, float32r matmul mode: full PE rate when free dim >= 256.
    return ap.bitcast(F32R)


def _build_module():
    nc = bacc.Bacc("TRN2", target_bir_lowering=False, debug=False, num_devices=8)

    x_t = nc.dram_tensor("x", [C, N], BF16, kind="ExternalInput")
    w_t = nc.dram_tensor("wT", [4, C, C], BF16, kind="ExternalInput")  # q,k,v,p (transposed)
    b_t = nc.dram_tensor("b", [2, C], F32, kind="ExternalInput")      # bq, bk
    bv_t = nc.dram_tensor("bv", [1, C], F32, kind="ExternalInput")
    y_t = nc.dram_tensor("y", [C, N], F32, kind="ExternalOutput")

    with tile.TileContext(nc) as tc:
        _emit(nc, tc, x_t, w_t, b_t, bv_t, y_t)
    nc.compile()
    return nc


def _emit(nc, tc, x_t, w_t, b_t, bv_t, y_t):
    from contextlib import ExitStack

    with ExitStack() as top:
        const = top.enter_context(tc.tile_pool(name="const", bufs=1))
        big = top.enter_context(tc.tile_pool(name="big", bufs=1))

        # ---- constants (packed to dodge the 4KB alloc granularity) -----
        # w_all[:, 2*w + ci, :] = rows ci*128.. of weight w's transpose [ci, co]
        w_all = const.tile([P, 8, C], BF16, tag="w_all", name="w_all")
        for w in range(4):
            for ci in range(2):
                nc.sync.dma_start(
                    w_all[:, 2 * w + ci, :],
                    w_t.ap()[w, ci * P:(ci + 1) * P, :],
                )

        def wslice(w, ci, ch):   # lhsT [128 ci, 128 co] for co half ch
            return w_all[:, 2 * w + ci, ch * P:(ch + 1) * P]

        # b_all columns: 0,1 = bq halves; 2,3 = bk halves
        b_all = const.tile([P, 4], F32, tag="b_all", name="b_all")
        for w in range(2):
            for ch in range(2):
                nc.sync.dma_start(
                    b_all[:, 2 * w + ch: 2 * w + ch + 1],
                    b_t.ap()[w, ch * P:(ch + 1) * P].rearrange("(p o) -> p o", o=1),
                )
        bv_sb = const.tile([P, C], F32, tag="bv", name="bv_sb")
        nc.sync.dma_start(
            bv_sb[:], bass.AP(tensor=bv_t, offset=0, ap=[[0, P], [1, C]])
        )

        # ---- persistent activations -----------------------------------
        q_bf = [big.tile([P, N], BF16, tag=f"q{ch}", name=f"q{ch}") for ch in range(2)]
        k_bf = [big.tile([P, NJ], BF16, tag=f"k{ch}", name=f"k{ch}") for ch in range(2)]
        v_all = big.tile([P, NJT, C], BF16, tag="v_all", name="v_all")
        attn = [big.tile([P, N], BF16, tag=f"a{jt}", name=f"a{jt}") for jt in range(NJT)]
        # d_all columns: 0:32 = per-(jt,ih) partial sums, 32:48 = D, 48:64 = 1/D
        d_all = big.tile([P, 64], F32, tag="d_all", name="d_all")

        # ---- phase 1: QKV projections ---------------------------------
        with tc.tile_pool(name="xload", bufs=1) as xp, \
             tc.tile_pool(name="ps_qkv", bufs=8, space="PSUM") as pq:
            x_sb = []
            for ci in range(2):
                xt = xp.tile([P, N], BF16, tag=f"x{ci}", name=f"x{ci}")
                nc.sync.dma_start(xt[:], x_t.ap()[ci * P:(ci + 1) * P, :])
                x_sb.append(xt)

            # q[co, i] = wqT.T @ x  (+bq) -> bf16
            for ch in range(2):
                for icg in range(2):
                    pss = [pq.tile([P, 512], F32, tag="ps", name="ps") for _ in range(4)]
                    for ci in range(2):
                        lhs = wslice(0, ci, ch)
                        for t in range(4):
                            ic = icg * 4 + t
                            nc.tensor.matmul(
                                pss[t][:], lhs,
                                x_sb[ci][:, ic * 512:(ic + 1) * 512],
                                start=(ci == 0), stop=(ci == 1),
                            )
                    for t in range(4):
                        ic = icg * 4 + t
                        nc.scalar.activation(
                            q_bf[ch][:, ic * 512:(ic + 1) * 512], pss[t][:],
                            AF.Identity, bias=b_all[:, ch:ch + 1],
                        )

            # k[co, j] over columns 0..NJ (+bk) -> bf16
            for ch in range(2):
                pss = [pq.tile([P, 512], F32, tag="ps", name="ps") for _ in range(4)]
                for ci in range(2):
                    lhs = wslice(1, ci, ch)
                    for t in range(4):
                        nc.tensor.matmul(
                            pss[t][:], lhs,
                            x_sb[ci][:, t * 512:(t + 1) * 512],
                            start=(ci == 0), stop=(ci == 1),
                        )
                for t in range(4):
                    nc.scalar.activation(
                        k_bf[ch][:, t * 512:(t + 1) * 512], pss[t][:],
                        AF.Identity, bias=b_all[:, 2 + ch:3 + ch],
                    )

            # v_T[j, co] = x[:, :NJ]^T @ wvT (+bv broadcast) -> bf16
            for jtg in range(4):
                pss = [pq.tile([P, C], F32, tag="ps", name="ps") for _ in range(4)]
                for ci in range(2):
                    for t in range(4):
                        jt = jtg * 4 + t
                        nc.tensor.matmul(
                            pss[t][:],
                            x_sb[ci][:, jt * P:(jt + 1) * P],
                            w_all[:, 2 * 2 + ci, :],
                            start=(ci == 0), stop=(ci == 1),
                        )
                for t in range(4):
                    nc.vector.tensor_add(
                        v_all[:, jtg * 4 + t, :], pss[t][:], bv_sb[:]
                    )

        # ---- phase 2: scores + exp + denominators ---------------------
        with tc.tile_pool(name="ps_s", bufs=2, space="PSUM") as psp:
            for jt in range(NJT):
                for ih in range(2):
                    ps = psp.tile([P, 2048], F32, tag="s", name="s_ps")
                    for ch in range(2):
                        lhs = k_bf[ch][:, jt * P:(jt + 1) * P]
                        for t in range(4):
                            nc.tensor.matmul(
                                ps[:, t * 512:(t + 1) * 512], lhs,
                                q_bf[ch][:, ih * 2048 + t * 512: ih * 2048 + (t + 1) * 512],
                                start=(ch == 0), stop=(ch == 1),
                            )
                    col = jt * 2 + ih
                    nc.scalar.activation(
                        attn[jt][:, ih * 2048:(ih + 1) * 2048], ps[:],
                        AF.Exp, scale=float(SCALE),
                        accum_out=d_all[:, col:col + 1],
                    )

        # per-j denominator: D = sum_i exp(s); fold 1/D into v rows
        nc.vector.reduce_sum(
            d_all[:, 32:48], d_all[:, 0:32].rearrange("p (a b) -> p a b", b=2),
            axis=mybir.AxisListType.X,
        )
        nc.vector.reciprocal(d_all[:, 48:64], d_all[:, 32:48])
        for jt in range(NJT):
            nc.vector.tensor_scalar_mul(
                v_all[:, jt, :], v_all[:, jt, :], d_all[:, 48 + jt:49 + jt]
            )

        # ---- phase 3: out = v_scaled @ attn ; y = wp @ out -------------
        with tc.tile_pool(name="ps_o", bufs=1, space="PSUM") as po, \
             tc.tile_pool(name="ps_y", bufs=1, space="PSUM") as py, \
             tc.tile_pool(name="osb", bufs=2) as osb_pool, \
             tc.tile_pool(name="ysb", bufs=2) as ysb_pool:
            for iq in range(4):
                o_ps = [po.tile([P, 1024], F32, tag=f"o{ch}", name=f"o{ch}") for ch in range(2)]
                for jt in range(NJT):
                    for ch in range(2):
                        lhs = v_all[:, jt, ch * P:(ch + 1) * P]
                        for t in range(2):
                            nc.tensor.matmul(
                                o_ps[ch][:, t * 512:(t + 1) * 512], lhs,
                                attn[jt][:, iq * 1024 + t * 512: iq * 1024 + (t + 1) * 512],
                                start=(jt == 0), stop=(jt == NJT - 1),
                            )
                o_sb = [osb_pool.tile([P, 1024], BF16, tag=f"osb{ch}", name=f"osb{ch}") for ch in range(2)]
                for ch in range(2):
                    nc.scalar.copy(o_sb[ch][:], o_ps[ch][:])
                for cho in range(2):
                    y_ps = py.tile([P, 1024], F32, tag=f"y{cho}", name=f"y{cho}")
                    for ci in range(2):
                        lhs = wslice(3, ci, cho)
                        for t in range(2):
                            nc.tensor.matmul(
                                y_ps[:, t * 512:(t + 1) * 512], lhs,
                                o_sb[ci][:, t * 512:(t + 1) * 512],
                                start=(ci == 0), stop=(ci == 1),
                            )
                    y_sb = ysb_pool.tile([P, 1024], F32, tag=f"ysb{cho}", name=f"ysb{cho}")
                    nc.vector.tensor_copy(y_sb[:], y_ps[:])
                    nc.sync.dma_start(
                        y_t.ap()[cho * P:(cho + 1) * P, iq * 1024:(iq + 1) * 1024],
                        y_sb[:],
                    )


_nc_cache = None
LAST_EXEC_TIME_NS = None


def _get_nc():
    global _nc_cache
    if _nc_cache is None:
        _nc_cache = _build_module()
    return _nc_cache


def kernel(x, wq, bq, wk, bk, wv, bv, wp, bp):
    global LAST_EXEC_TIME_NS
    nc = _get_nc()

    import ml_dtypes
    bf = ml_dtypes.bfloat16
    x = np.asarray(x, dtype=np.float32).reshape(B, C, N).astype(bf)
    wT = np.ascontiguousarray(np.stack([
        np.asarray(w, dtype=np.float32).T for w in (wq, wk, wv, wp)
    ])).astype(bf)
    b2 = np.ascontiguousarray(np.stack([
        np.asarray(bq, dtype=np.float32), np.asarray(bk, dtype=np.float32)
    ]))
    bv2 = np.ascontiguousarray(np.asarray(bv, dtype=np.float32).reshape(1, C))
    bp1 = np.asarray(bp, dtype=np.float32).reshape(C)

    in_maps = []
    for core in range(8):
        b, h = divmod(core, 2)
        xb = x[b] if h == 0 else np.ascontiguousarray(np.roll(x[b], -NJ, axis=1))
        in_maps.append({"x": xb, "wT": wT, "b": b2, "bv": bv2})

    res = bass_utils.run_bass_kernel_spmd(nc, in_maps, core_ids=list(range(8)))
    if res.exec_time_ns is not None:
        LAST_EXEC_TIME_NS = res.exec_time_ns

    y = np.zeros((B, C, N), np.float32)
    for b in range(B):
        y[b] = res.results[2 * b]["y"] + np.roll(res.results[2 * b + 1]["y"], NJ, axis=1)
    y += bp1.reshape(1, C, 1)
    return y.reshape(B, C, 64, 64)


# revision 7
# speedup vs baseline: 1.0460x; 1.0171x over previous
"""Trainium2 Bass kernel for the AttnBlock-style attention module.

Reference computation (note softmax over axis=1, the *i* axis):
    q = wq @ x + bq ; k = wk @ x + bk ; v = wv @ x + bv      (per-pixel 1x1 conv)
    s[b,i,j] = (q[b,:,i] . k[b,:,j]) * C**-0.5
    attn = softmax_i(s)                                      (normalize over i!)
    out[b,c,i] = sum_j attn[b,i,j] v[b,c,j]
    y = wp @ out + bp

Sharding: 8 cores = 4 batches x 2 j-halves. The softmax over i is local to a
j-split (it normalizes each attention *column* j over all i). Each core gets x
with its j-half rotated to columns 0..2047 (a pure permutation of the pixel
axis, which passes through every per-pixel op and the i-softmax unchanged; the
host un-rotates the partial output). Each core:
  - computes q for all N=4096 pixels, k/v for columns 0..2047,
  - s_T[j, i] = k^T q   (j on partitions -> softmax reduction is free-axis),
  - attn = exp(s/16) stored unnormalized in bf16; per-j denominators D[j]
    from the fused activation accum_out; 1/D folded into v rows,
  - out_partial[c, i] = sum_{j in half} v_scaled[c,j] attn_T[j,i],
  - y_partial = wp @ out_partial   (bias bp added on host).
Host un-rotates and sums the two j-half partials per batch and adds bp.
"""

import numpy as np

import concourse.bass as bass
import concourse.mybir as mybir
import concourse.tile as tile
from concourse import bacc
from concourse import bass_utils

P = 128
B = 4
C = 256
N = 4096          # 64*64 pixels
NJ = 2048         # j columns per core
NJT = NJ // P     # 16 j tiles
SCALE = 1.0 / np.sqrt(C).item()   # 1/16

F32 = mybir.dt.float32
BF16 = mybir.dt.bfloat16
F32R = mybir.dt.float32r
AF = mybir.ActivationFunctionType


def _fr(ap):
    # fp32 data, float32r matmul mode: full PE rate when free dim >= 256.
    return ap.bitcast(F32R)


def _build_module():
    nc = bacc.Bacc("TRN2", target_bir_lowering=False, debug=False, num_devices=8)

    x_t = nc.dram_tensor("x", [C, N], BF16, kind="ExternalInput")
    w_t = nc.dram_tensor("wT", [4, C, C], BF16, kind="ExternalInput")  # q,k,v,p (transposed)
    b_t = nc.dram_tensor("b", [2, C], F32, kind="ExternalInput")      # bq, bk
    bv_t = nc.dram_tensor("bv", [1, C], F32, kind="ExternalInput")
    y_t = nc.dram_tensor("y", [C, N], F32, kind="ExternalOutput")

    with tile.TileContext(nc) as tc:
        _emit(nc, tc, x_t, w_t, b_t, bv_t, y_t)
    nc.compile()
    return nc


def _emit(nc, tc, x_t, w_t, b_t, bv_t, y_t):
    from contextlib import ExitStack

    with ExitStack() as top:
        const = top.enter_context(tc.tile_pool(name="const", bufs=1))
        big = top.enter_context(tc.tile_pool(name="big", bufs=1))

        # ---- constants (packed to dodge the 4KB alloc granularity) -----
        # w_all[:, 2*w + ci, :] = rows ci*128.. of weight w's transpose [ci, co]
        w_all = const.tile([P, 8, C], BF16, tag="w_all", name="w_all")
        for w in range(4):
            for ci in range(2):
                nc.sync.dma_start(
                    w_all[:, 2 * w + ci, :],
                    w_t.ap()[w, ci * P:(ci + 1) * P, :],
                )

        def wslice(w, ci, ch):   # lhsT [128 ci, 128 co] for co half ch
            return w_all[:, 2 * w + ci, ch * P:(ch + 1) * P]

        # b_all columns: 0,1 = bq halves; 2,3 = bk halves
        b_all = const.tile([P, 4], F32, tag="b_all", name="b_all")
        for w in range(2):
            for ch in range(2):
                nc.sync.dma_start(
                    b_all[:, 2 * w + ch: 2 * w + ch + 1],
                    b_t.ap()[w, ch * P:(ch + 1) * P].rearrange("(p o) -> p o", o=1),
                )
        bv_sb = const.tile([P, C], F32, tag="bv", name="bv_sb")
        nc.sync.dma_start(
            bv_sb[:], bass.AP(tensor=bv_t, offset=0, ap=[[0, P], [1, C]])
        )

        # ---- persistent activations -----------------------------------
        q_bf = [big.tile([P, N], BF16, tag=f"q{ch}", name=f"q{ch}") for ch in range(2)]
        k_bf = [big.tile([P, NJ], BF16, tag=f"k{ch}", name=f"k{ch}") for ch in range(2)]
        v_all = big.tile([P, NJT, C], BF16, tag="v_all", name="v_all")
        attn = [big.tile([P, N], BF16, tag=f"a{jt}", name=f"a{jt}") for jt in range(NJT)]
        # d_all columns: 0:32 = per-(jt,ih) partial sums, 32:48 = D, 48:64 = 1/D
        d_all = big.tile([P, 64], F32, tag="d_all", name="d_all")

        # ---- phase 1: QKV projections ---------------------------------
        # x arrives in [128, 1024] column blocks so matmuls start early;
        # k and v (which only need columns 0..NJ) are computed before q.
        with tc.tile_pool(name="xload", bufs=1) as xp, \
             tc.tile_pool(name="ps_qkv", bufs=8, space="PSUM") as pq:
            x_sb = [xp.tile([P, N], BF16, tag=f"x{ci}", name=f"x{ci}")
                    for ci in range(2)]
            for blk in range(4):
                for ci in range(2):
                    nc.sync.dma_start(
                        x_sb[ci][:, blk * 1024:(blk + 1) * 1024],
                        x_t.ap()[ci * P:(ci + 1) * P, blk * 1024:(blk + 1) * 1024],
                    )

            def bias_store(out_ap, ps, bias_ap, on_act):
                if on_act:
                    nc.scalar.activation(out_ap, ps, AF.Identity, bias=bias_ap)
                else:
                    nc.vector.tensor_scalar_add(out_ap, ps, bias_ap)

            # k[co, j] over columns 0..NJ (+bk) -> bf16
            for ch in range(2):
                pss = [pq.tile([P, 512], F32, tag="ps", name="ps") for _ in range(4)]
                for ci in range(2):
                    lhs = wslice(1, ci, ch)
                    for t in range(4):
                        nc.tensor.matmul(
                            pss[t][:], lhs,
                            x_sb[ci][:, t * 512:(t + 1) * 512],
                            start=(ci == 0), stop=(ci == 1),
                        )
                for t in range(4):
                    bias_store(k_bf[ch][:, t * 512:(t + 1) * 512], pss[t][:],
                               b_all[:, 2 + ch:3 + ch], on_act=(ch == 0))

            # v_T[j, co] = x[:, :NJ]^T @ wvT (+bv broadcast) -> bf16
            for jtg in range(4):
                pss = [pq.tile([P, C], F32, tag="ps", name="ps") for _ in range(4)]
                for ci in range(2):
                    for t in range(4):
                        jt = jtg * 4 + t
                        nc.tensor.matmul(
                            pss[t][:],
                            x_sb[ci][:, jt * P:(jt + 1) * P],
                            w_all[:, 2 * 2 + ci, :],
                            start=(ci == 0), stop=(ci == 1),
                        )
                for t in range(4):
                    nc.vector.tensor_add(
                        v_all[:, jtg * 4 + t, :], pss[t][:], bv_sb[:]
                    )

            # q[co, i] = wqT.T @ x  (+bq) -> bf16
            for icg in range(2):
                for ch in range(2):
                    pss = [pq.tile([P, 512], F32, tag="ps", name="ps") for _ in range(4)]
                    for ci in range(2):
                        lhs = wslice(0, ci, ch)
                        for t in range(4):
                            ic = icg * 4 + t
                            nc.tensor.matmul(
                                pss[t][:], lhs,
                                x_sb[ci][:, ic * 512:(ic + 1) * 512],
                                start=(ci == 0), stop=(ci == 1),
                            )
                    for t in range(4):
                        ic = icg * 4 + t
                        bias_store(q_bf[ch][:, ic * 512:(ic + 1) * 512], pss[t][:],
                                   b_all[:, ch:ch + 1], on_act=(ch == 0))

        # ---- phase 2: scores + exp + denominators ---------------------
        with tc.tile_pool(name="ps_s", bufs=2, space="PSUM") as psp:
            for jt in range(NJT):
                for ih in range(2):
                    ps = psp.tile([P, 2048], F32, tag="s", name="s_ps")
                    for ch in range(2):
                        lhs = k_bf[ch][:, jt * P:(jt + 1) * P]
                        for t in range(4):
                            nc.tensor.matmul(
                                ps[:, t * 512:(t + 1) * 512], lhs,
                                q_bf[ch][:, ih * 2048 + t * 512: ih * 2048 + (t + 1) * 512],
                                start=(ch == 0), stop=(ch == 1),
                            )
                    nc.scalar.activation(
                        attn[jt][:, ih * 2048:(ih + 1) * 2048], ps[:],
                        AF.Exp, scale=float(SCALE),
                    )
                if True:
                    # per-jt denominator + v scaling on DVE, so the out phase
                    # never waits on a global reduction over all j tiles
                    nc.vector.reduce_sum(
                        d_all[:, jt:jt + 1], attn[jt][:],
                        axis=mybir.AxisListType.X,
                    )
                    nc.vector.reciprocal(
                        d_all[:, 32 + jt:33 + jt], d_all[:, jt:jt + 1]
                    )
                    nc.vector.tensor_scalar_mul(
                        v_all[:, jt, :], v_all[:, jt, :],
                        d_all[:, 32 + jt:33 + jt],
                    )

        # ---- phase 3: out = v_scaled @ attn ; y = wp @ out -------------
        with tc.tile_pool(name="ps_o", bufs=1, space="PSUM") as po, \
             tc.tile_pool(name="ps_y", bufs=1, space="PSUM") as py, \
             tc.tile_pool(name="osb", bufs=2) as osb_pool, \
             tc.tile_pool(name="ysb", bufs=2) as ysb_pool:
            for iq in range(4):
                o_ps = [po.tile([P, 1024], F32, tag=f"o{ch}", name=f"o{ch}") for ch in range(2)]
                for jt in range(NJT):
                    for ch in range(2):
                        lhs = v_all[:, jt, ch * P:(ch + 1) * P]
                        for t in range(2):
                            nc.tensor.matmul(
                                o_ps[ch][:, t * 512:(t + 1) * 512], lhs,
                                attn[jt][:, iq * 1024 + t * 512: iq * 1024 + (t + 1) * 512],
                                start=(jt == 0), stop=(jt == NJT - 1),
                            )
                o_sb = [osb_pool.tile([P, 1024], BF16, tag=f"osb{ch}", name=f"osb{ch}") for ch in range(2)]
                for ch in range(2):
                    nc.scalar.copy(o_sb[ch][:], o_ps[ch][:])
                for cho in range(2):
                    y_ps = py.tile([P, 1024], F32, tag=f"y{cho}", name=f"y{cho}")
                    for ci in range(2):
                        lhs = wslice(3, ci, cho)
                        for t in range(2):
                            nc.tensor.matmul(
                                y_ps[:, t * 512:(t + 1) * 512], lhs,
                                o_sb[ci][:, t * 512:(t + 1) * 512],
                                start=(ci == 0), stop=(ci == 1),
                            )
                    y_sb = ysb_pool.tile([P, 1024], F32, tag=f"ysb{cho}", name=f"ysb{cho}")
                    nc.vector.tensor_copy(y_sb[:], y_ps[:])
                    nc.sync.dma_start(
                        y_t.ap()[cho * P:(cho + 1) * P, iq * 1024:(iq + 1) * 1024],
                        y_sb[:],
                    )


_nc_cache = None
LAST_EXEC_TIME_NS = None


def _get_nc():
    global _nc_cache
    if _nc_cache is None:
        _nc_cache = _build_module()
    return _nc_cache


def kernel(x, wq, bq, wk, bk, wv, bv, wp, bp):
    global LAST_EXEC_TIME_NS
    nc = _get_nc()

    import ml_dtypes
    bf = ml_dtypes.bfloat16
    x = np.asarray(x, dtype=np.float32).reshape(B, C, N).astype(bf)
    wT = np.ascontiguousarray(np.stack([
        np.asarray(w, dtype=np.float32).T for w in (wq, wk, wv, wp)
    ])).astype(bf)
    b2 = np.ascontiguousarray(np.stack([
        np.asarray(bq, dtype=np.float32), np.asarray(bk, dtype=np.float32)
    ]))
    bv2 = np.ascontiguousarray(np.asarray(bv, dtype=np.float32).reshape(1, C))
    bp1 = np.asarray(bp, dtype=np.float32).reshape(C)

    in_maps = []
    for core in range(8):
        b, h = divmod(core, 2)
        xb = x[b] if h == 0 else np.ascontiguousarray(np.roll(x[b], -NJ, axis=1))
        in_maps.append({"x": xb, "wT": wT, "b": b2, "bv": bv2})

    res = bass_utils.run_bass_kernel_spmd(nc, in_maps, core_ids=list(range(8)))
    if res.exec_time_ns is not None:
        LAST_EXEC_TIME_NS = res.exec_time_ns

    y = np.zeros((B, C, N), np.float32)
    for b in range(B):
        y[b] = res.results[2 * b]["y"] + np.roll(res.results[2 * b + 1]["y"], NJ, axis=1)
    y += bp1.reshape(1, C, 1)
    return y.reshape(B, C, 64, 64)
